# revision 1
# baseline (speedup 1.0000x reference)
"""Trainium2 Bass kernel for nn_Clusterer loss (Concrete-mixture clustering loss).

Strategy (data-parallel over N across 8 cores, per sharding hint):
  - All heavy per-row work (N x K = 262144 x 64) on device:
      v = z + logN computed by ONE fp16 matmul per 128-row tile
        (stationary operand = [x^T; x2; 1; z^T] feature pack, moving operand
         = [w; a; cc; I64] built from mu/r on host)
      row-wise logsumexp over K of v (max on DVE, exp on ACT, sum on DVE)
      con-side sums (sum_k e^z, sum_k pi_k e^{-tau z}, sum_k z) via PE
        matmuls over host-transposed z (2-up, 128 partitions), with a
        sliding-window selector matrix routing each chunk's sums to its own
        PSUM partition rows.
  - Tiny K/D-sized losses (pi/mu/lambda/b/r/C) + final reduction on host in
    float64 (exact mirror of the reference formulas).
"""

import math
import os

import numpy as np

N, D, K = 262144, 16, 64
NCORES = 8
NS = N // NCORES          # rows per core = 32768
NG = NS // 128            # 128-row groups per core = 256
G_SC = 16                 # groups per super-chunk
N_SC = NG // G_SC         # super-chunks = 16
FD_SC = G_SC * 64         # rows-side free dim per SC = 1024
TCHUNK = 512              # zTp columns per sums-matmul chunk (= 1024 rows)
NCHUNK = (NS // 2) // TCHUNK  # = 32 (must be <= 32 so 2*NCHUNK <= 64 psum rows)
TAU = 0.1
LOG2PI = math.log(2.0 * math.pi)

_cache = {}


def _build_program():
    import concourse.bacc as bacc
    import concourse.mybir as mybir
    import concourse.tile as tile

    fp16 = mybir.dt.float16
    fp32 = mybir.dt.float32
    AF = mybir.ActivationFunctionType
    ALU = mybir.AluOpType
    AX = mybir.AxisListType

    nc = bacc.Bacc("TRN2", target_bir_lowering=False, debug=False,
                   num_devices=NCORES)

    lpack = nc.dram_tensor("lpack", [128, NS], fp16, kind="ExternalInput").ap()
    ztp = nc.dram_tensor("ztp", [128, NS // 2], fp16, kind="ExternalInput").ap()
    rhsv = nc.dram_tensor("rhsv", [128, 64], fp16, kind="ExternalInput").ap()
    selw = nc.dram_tensor("selw", [128, 192], fp16, kind="ExternalInput").ap()
    lnpi = nc.dram_tensor("lnpi", [128, 1], fp32, kind="ExternalInput").ap()
    out_parts = nc.dram_tensor("out_parts", [128, 2], fp32,
                               kind="ExternalOutput").ap()

    with tile.TileContext(nc) as tc:
        with (
            tc.tile_pool(name="const", bufs=1) as constp,
            tc.tile_pool(name="stats", bufs=1) as statp,
            tc.tile_pool(name="lp", bufs=3) as lpp,
            tc.tile_pool(name="zt", bufs=4) as ztpp,
            tc.tile_pool(name="ex", bufs=4) as exp_pool,
            tc.tile_pool(name="vs", bufs=2) as vsp,
            tc.tile_pool(name="eu", bufs=2) as eup,
            tc.tile_pool(name="ep", bufs=1) as epp,
            tc.tile_pool(name="vps", bufs=2, space="PSUM") as vpsp,
            tc.tile_pool(name="sps", bufs=1, space="PSUM") as spsp,
        ):
            rhsv_t = constp.tile([128, 64], fp16, tag="rhsv")
            nc.sync.dma_start(rhsv_t[:], rhsv[:])
            selw_t = constp.tile([128, 192], fp16, tag="selw")
            nc.sync.dma_start(selw_t[:], selw[:])
            lnpi_t = constp.tile([128, 1], fp32, tag="lnpi")
            nc.sync.dma_start(lnpi_t[:], lnpi[:])

            mu_all = statp.tile([128, NG], fp32, tag="mu_all")
            su_all = statp.tile([128, NG], fp32, tag="su_all")

            sz_ps = spsp.tile([64, TCHUNK], fp32, tag="sz")
            st_ps = spsp.tile([64, TCHUNK], fp32, tag="st")
            zs_ps = spsp.tile([64, TCHUNK], fp32, tag="zs")

            for sc in range(N_SC):
                # ---- rows-side: v = z + logN via per-tile matmuls ----
                lp_t = lpp.tile([128, G_SC * 128], fp16, tag="lp")
                nc.sync.dma_start(
                    lp_t[:], lpack[:, sc * G_SC * 128:(sc + 1) * G_SC * 128])
                vps = vpsp.tile([128, FD_SC], fp32, tag="v")
                for g in range(G_SC):
                    nc.tensor.matmul(
                        vps[:, g * 64:(g + 1) * 64],
                        lhsT=lp_t[:, g * 128:(g + 1) * 128],
                        rhs=rhsv_t[:],
                        start=True, stop=True,
                    )
                v3 = vps[:].rearrange("p (g k) -> p g k", k=64)
                mu_sl = mu_all[:, sc * G_SC:(sc + 1) * G_SC]
                nc.vector.reduce_max(mu_sl, v3, axis=AX.X)
                vs_t = vsp.tile([128, FD_SC], fp32, tag="vs")
                mu_b = mu_sl.broadcast_to([128, G_SC, 64])
                nc.vector.scalar_tensor_tensor(
                    vs_t[:].rearrange("p (g k) -> p g k", k=64),
                    in0=v3, scalar=1.0, in1=mu_b,
                    op0=ALU.mult, op1=ALU.subtract)
                eu_t = eup.tile([128, FD_SC], fp16, tag="eu")
                nc.scalar.activation(eu_t[:], vs_t[:], AF.Exp)
                nc.vector.reduce_sum(
                    su_all[:, sc * G_SC:(sc + 1) * G_SC],
                    eu_t[:].rearrange("p (g k) -> p g k", k=64), axis=AX.X)

                # ---- con-side: transposed-z sums via PE ----
                zt_t = ztpp.tile([128, 2 * TCHUNK], fp16, tag="zt")
                nc.sync.dma_start(
                    zt_t[:], ztp[:, sc * 2 * TCHUNK:(sc + 1) * 2 * TCHUNK])
                for h in range(2):
                    c = sc * 2 + h
                    zt_c = zt_t[:, h * TCHUNK:(h + 1) * TCHUNK]
                    e1_t = exp_pool.tile([128, TCHUNK], fp16, tag="e1")
                    nc.scalar.activation(e1_t[:], zt_c, AF.Exp)
                    e2_t = exp_pool.tile([128, TCHUNK], fp16, tag="e2")
                    nc.scalar.activation(e2_t[:], zt_c, AF.Exp,
                                         bias=lnpi_t[:, 0:1], scale=-TAU)
                    sel = selw_t[:, 64 - 2 * c:128 - 2 * c]
                    first = (c == 0)
                    last = (c == NCHUNK - 1)
                    nc.tensor.matmul(sz_ps[:], lhsT=sel, rhs=e1_t[:],
                                     start=first, stop=last)
                    nc.tensor.matmul(st_ps[:], lhsT=sel, rhs=e2_t[:],
                                     start=first, stop=last)
                    nc.tensor.matmul(zs_ps[:], lhsT=sel, rhs=zt_c,
                                     start=first, stop=last)

            # ---- epilogue ----
            # A-side (con, [64, TCHUNK]): -1.1*sumz + 63*ln(sz) - 64*ln(st)
            lnsz = epp.tile([64, TCHUNK], fp32, tag="lnsz")
            nc.scalar.activation(lnsz[:], sz_ps[:], AF.Ln)
            lnst = epp.tile([64, TCHUNK], fp32, tag="lnst")
            nc.scalar.activation(lnst[:], st_ps[:], AF.Ln)
            acc_a = epp.tile([64, TCHUNK], fp32, tag="acca")
            nc.vector.scalar_tensor_tensor(
                acc_a[:], in0=lnst[:], scalar=-64.0 / 63.0, in1=lnsz[:],
                op0=ALU.mult, op1=ALU.add)
            acc_b = epp.tile([64, TCHUNK], fp32, tag="accb")
            nc.vector.scalar_tensor_tensor(
                acc_b[:], in0=zs_ps[:], scalar=-1.1 / 63.0, in1=acc_a[:],
                op0=ALU.mult, op1=ALU.add)
            a_part = epp.tile([64, 1], fp32, tag="apart")
            nc.vector.reduce_sum(a_part[:], acc_b[:], axis=AX.X)

            # B-side (mix, [128, NG]): m_u + ln(su)
            lnsu = epp.tile([128, NG], fp32, tag="lnsu")
            nc.scalar.activation(lnsu[:], su_all[:], AF.Ln)
            tot_b = epp.tile([128, NG], fp32, tag="totb")
            nc.vector.tensor_add(tot_b[:], lnsu[:], mu_all[:])
            out_t = epp.tile([128, 2], fp32, tag="outt")
            nc.vector.memset(out_t[:], 0.0)
            nc.vector.reduce_sum(out_t[:, 0:1], tot_b[:], axis=AX.X)
            nc.vector.tensor_scalar_mul(out_t[0:64, 1:2], a_part[:], 63.0)
            nc.sync.dma_start(out_parts[:], out_t[:])

    nc.compile()
    return nc


def _prep_inputs(met_locs, mu, pi, lambda_mu, b, C, r, z):
    """Host-side packing. Returns (in_maps, host_ctx)."""
    f64 = np.float64
    mu64 = mu.astype(f64)
    r64 = r.astype(f64)
    pi64 = pi.astype(f64)

    # per-k constants
    a = -0.5 * np.exp(-r64)                       # [K]
    mu2 = (mu64 ** 2).sum(1)                      # [K]
    ck = -0.5 * D * (r64 + LOG2PI)                # [K]
    cck = a * mu2 + ck                            # [K]
    # log_softmax(pi) in f64:
    m = pi64.max()
    lnpi64 = pi64 - (m + np.log(np.exp(pi64 - m).sum()))

    # hi/lo split of the per-k constants (a_k, cck): their fp16 rounding is
    # systematic across all N rows, so carry the residual on a second
    # contraction row (rows 16/19 multiply x2, rows 17/18 multiply 1).
    rhsv = np.zeros((128, 64), np.float16)
    rhsv[0:16, :] = (-2.0 * a[None, :] * mu64.T).astype(np.float16)
    a_hi = a.astype(np.float16)
    rhsv[16, :] = a_hi
    cck_hi = cck.astype(np.float16)
    rhsv[17, :] = cck_hi
    rhsv[18, :] = (cck - cck_hi.astype(f64)).astype(np.float16)
    rhsv[19, :] = (a - a_hi.astype(f64)).astype(np.float16)
    rhsv[20, :] = a_hi                     # multiplies the x2 fp16 residual
    rhsv[32:96, :] = np.eye(64, dtype=np.float16)

    selw = np.zeros((128, 192), np.float16)
    selw[0:64, 64] = 1.0
    selw[64:128, 65] = 1.0

    lnpi32 = np.zeros((128, 1), np.float32)
    lnpi32[0:64, 0] = lnpi64.astype(np.float32)
    lnpi32[64:128, 0] = lnpi64.astype(np.float32)

    in_maps = []
    for i in range(NCORES):
        rs = slice(i * NS, (i + 1) * NS)
        xc = met_locs[rs]                          # [NS, 16] fp32
        zc = z[rs]                                 # [NS, 64] fp32
        x2c = (xc.astype(f64) ** 2).sum(1)

        lpack = np.zeros((128, NS), np.float16)
        lpack[0:16, :] = xc.T.astype(np.float16)
        x2_hi = x2c.astype(np.float16)
        lpack[16, :] = x2_hi
        lpack[17, :] = 1.0
        lpack[18, :] = 1.0                      # carries cck_lo
        lpack[19, :] = x2_hi                    # carries a_lo
        # x2 fp16 residual enters via the a_k row in fp16-sized pieces:
        lpack[20, :] = (x2c - x2_hi.astype(f64)).astype(np.float16)
        lpack[32:96, :] = zc.T.astype(np.float16)

        zr = zc.reshape(NS // 2, 2, 64)
        ztp = np.concatenate(
            [np.ascontiguousarray(zr[:, 0, :].T),
             np.ascontiguousarray(zr[:, 1, :].T)], axis=0).astype(np.float16)

        in_maps.append({
            "lpack": np.ascontiguousarray(lpack),
            "ztp": np.ascontiguousarray(ztp),
            "rhsv": rhsv,
            "selw": selw,
            "lnpi": lnpi32,
        })

    const0 = (math.lgamma(float(K)) + (K - 1) * math.log(TAU)
              + float(lnpi64.sum()))
    return in_maps, {"const0": const0, "lnpi64": lnpi64}


def _host_small_losses(met_locs, mu, pi, lambda_mu, b, C, r, lnpi64):
    """All parameter-only losses in float64, mirroring the reference."""
    f64 = np.float64
    x64 = met_locs.astype(f64)
    R = x64.max(0) - x64.min(0)
    Df = float(D)
    c = 1.25 + (D - 1) / 4.0
    g = 0.25 + (D - 1) / 4.0
    G = c / (50.0 * g) * math.sqrt(float((R ** 2).sum()))

    pi_loss = -((1.0 / K - 1.0) * lnpi64).sum()

    lam = lambda_mu.astype(f64)
    var_mu = (lam ** 2) * R
    mu64 = mu.astype(f64)
    b64 = b.astype(f64)
    mu_lp = (-0.5 * (((mu64 - b64) ** 2) / var_mu[None, :]).sum(1)
             - 0.5 * np.log(var_mu).sum() - 0.5 * Df * LOG2PI)
    mu_loss = -mu_lp.sum()

    lam_lp = (0.5 * math.log(0.5) - math.lgamma(0.5)
              + (0.5 - 1.0) * lam - 0.5 * np.exp(lam))
    lambda_loss = -lam_lp.sum()

    b_loss = 0.5 * (b64 ** 2).sum() + 0.5 * K * Df * LOG2PI

    r64 = r.astype(f64)
    C64 = C.astype(f64)
    r_lp = (c * np.log(C64) + (c - 1.0) * (-r64) - C64 * np.exp(-r64)
            - math.lgamma(c))
    r_loss = -r_lp.sum()

    C_lp = (g * math.log(G) + (g - 1.0) * (-C64) - G * np.exp(-C64)
            - math.lgamma(g))
    C_loss = -C_lp.sum()

    return r_loss + mu_loss + pi_loss + b_loss + lambda_loss + C_loss


def kernel(met_locs, mu, pi, lambda_mu, b, C, r, z):
    from concourse import bass_utils

    met_locs = np.asarray(met_locs, dtype=np.float32)
    mu = np.asarray(mu, dtype=np.float32)
    pi = np.asarray(pi, dtype=np.float32)
    lambda_mu = np.asarray(lambda_mu, dtype=np.float32)
    b = np.asarray(b, dtype=np.float32)
    C = np.asarray(C, dtype=np.float32)
    r = np.asarray(r, dtype=np.float32)
    z = np.asarray(z, dtype=np.float32)

    if "nc" not in _cache:
        _cache["nc"] = _build_program()
    nc = _cache["nc"]

    in_maps, ctx = _prep_inputs(met_locs, mu, pi, lambda_mu, b, C, r, z)

    trace = bool(int(os.environ.get("KERNEL_TRACE", "0")))
    res = bass_utils.run_bass_kernel_spmd(
        nc, in_maps, core_ids=list(range(NCORES)), trace=trace)
    _cache["last_results"] = res

    con_mix = 0.0
    for cm in res.results:
        o = cm["out_parts"].astype(np.float64)
        con_mix += o[:, 0].sum() + o[0:64, 1].sum()
    con_mix += N * ctx["const0"]
    z_loss = -con_mix

    small = _host_small_losses(met_locs, mu, pi, lambda_mu, b, C, r,
                               ctx["lnpi64"])
    total = z_loss + small
    return np.asarray(total, dtype=np.float32)



# revision 3
# speedup vs baseline: 2.1064x; 2.1064x over previous
"""Trainium2 Bass kernel for nn_Clusterer loss (Concrete-mixture clustering loss).

Strategy (data-parallel over N across 8 cores, per sharding hint):
  Natural-layout design: z ships exactly once as fp16 [N, 64] (cast only, no
  host transpose); rows live on SBUF partitions, so every per-row reduction
  over K is a free-axis DVE/ACT reduction:
    v = z + logN:   PE matmul of a 21-row x-feature pack (x^T, x2 hi/lo, ones)
                    against per-k constants gives the logN part in PSUM; one
                    scalar_tensor_tensor adds z.
    row stats:      max_k v, sum e^{v-max}, sum e^z, sum pi_k e^{-tau z},
                    sum z -- all AX.X reduces into [128, NG] stat tiles.
  Per-core output = 4 partial sums [128, 4]; host combines in float64:
    con+mix = const0 + (M + ln su) + 63*ln sz - 64*ln st - 1.1*sum z.
  Tiny K/D-sized losses (pi/mu/lambda/b/r/C) computed on host in float64.
"""

import math
import os

import numpy as np

N, D, K = 262144, 16, 64
NCORES = 8
NS = N // NCORES          # rows per core = 32768
NG = NS // 128            # 128-row tiles per core = 256
G = 16                    # tiles per chunk
NCH = NG // G             # chunks = 16
FD = G * K                # free dim per chunk = 1024
TAU = 0.1
LOG2PI = math.log(2.0 * math.pi)

_cache = {}


def _build_program():
    import concourse.bacc as bacc
    import concourse.mybir as mybir
    import concourse.tile as tile

    fp16 = mybir.dt.float16
    fp32 = mybir.dt.float32
    AF = mybir.ActivationFunctionType
    ALU = mybir.AluOpType
    AX = mybir.AxisListType

    nc = bacc.Bacc("TRN2", target_bir_lowering=False, debug=False,
                   num_devices=NCORES)

    zt3 = nc.dram_tensor("zt3", [NG, 128, K], fp16, kind="ExternalInput").ap()
    lpk = nc.dram_tensor("lpk", [21, NS], fp16, kind="ExternalInput").ap()
    rhs = nc.dram_tensor("rhs", [21, K], fp16, kind="ExternalInput").ap()
    lnpik = nc.dram_tensor("lnpik", [128, K], fp32, kind="ExternalInput").ap()
    outp = nc.dram_tensor("outp", [128, 4], fp32, kind="ExternalOutput").ap()

    with tile.TileContext(nc) as tc:
        with (
            tc.tile_pool(name="const", bufs=1) as constp,
            tc.tile_pool(name="stats", bufs=1) as statp,
            tc.tile_pool(name="lp", bufs=3) as lpp,
            tc.tile_pool(name="zt", bufs=3) as ztp,
            tc.tile_pool(name="wk", bufs=2) as wkp,
            tc.tile_pool(name="ep", bufs=1) as epp,
            tc.tile_pool(name="ps", bufs=2, space="PSUM") as psp,
        ):
            rhs_t = constp.tile([21, K], fp16, tag="rhs")
            nc.sync.dma_start(rhs_t[:], rhs[:])
            lnpik_t = constp.tile([128, K], fp32, tag="lnpik")
            nc.sync.dma_start(lnpik_t[:], lnpik[:])
            # replicate lnpi along the chunk axis once: [128, G, K]
            lnpirep = constp.tile([128, G, K], fp32, tag="lnpirep")
            for g in range(G):
                nc.scalar.activation(lnpirep[:, g, :], lnpik_t[:], AF.Copy)

            mu_all = statp.tile([128, NG], fp32, tag="mu_all")
            su_all = statp.tile([128, NG], fp32, tag="su_all")
            sz_all = statp.tile([128, NG], fp32, tag="sz_all")
            st_all = statp.tile([128, NG], fp32, tag="st_all")
            zs_all = statp.tile([128, NG], fp32, tag="zs_all")

            for c in range(NCH):
                cols = slice(c * G, (c + 1) * G)
                lp_t = lpp.tile([21, G * 128], fp16, tag="lp")
                nc.sync.dma_start(
                    lp_t[:], lpk[:, c * G * 128:(c + 1) * G * 128])
                zt_t = ztp.tile([128, G, K], fp16, tag="zt")
                nc.sync.dma_start(
                    zt_t[:],
                    zt3[c * G:(c + 1) * G].rearrange("g p k -> p g k"))

                ps = psp.tile([128, FD], fp32, tag="ps")
                for g in range(G):
                    nc.tensor.matmul(
                        ps[:, g * K:(g + 1) * K],
                        lhsT=lp_t[:, g * 128:(g + 1) * 128],
                        rhs=rhs_t[:],
                        start=True, stop=True,
                    )
                ps3 = ps[:].rearrange("p (g k) -> p g k", k=K)

                # v = z + logN
                v = wkp.tile([128, G, K], fp32, tag="v")
                nc.vector.scalar_tensor_tensor(
                    v[:], in0=zt_t[:], scalar=1.0, in1=ps3,
                    op0=ALU.mult, op1=ALU.add)
                mu_sl = mu_all[:, cols]
                nc.vector.reduce_max(mu_sl, v[:], axis=AX.X)
                vc = wkp.tile([128, G, K], fp32, tag="vc")
                nc.vector.scalar_tensor_tensor(
                    vc[:], in0=v[:], scalar=1.0,
                    in1=mu_sl.broadcast_to([128, G, K]),
                    op0=ALU.mult, op1=ALU.subtract)
                e = wkp.tile([128, G, K], fp32, tag="e")
                nc.scalar.activation(e[:], vc[:], AF.Exp)
                nc.vector.reduce_sum(su_all[:, cols], e[:], axis=AX.X)

                # con-side sums from the same natural-layout z tile
                e1 = wkp.tile([128, G, K], fp32, tag="e1")
                nc.scalar.activation(e1[:], zt_t[:], AF.Exp)
                nc.vector.reduce_sum(sz_all[:, cols], e1[:], axis=AX.X)
                t3 = wkp.tile([128, G, K], fp32, tag="t3")
                nc.vector.scalar_tensor_tensor(
                    t3[:], in0=zt_t[:], scalar=-TAU, in1=lnpirep[:],
                    op0=ALU.mult, op1=ALU.add)
                e2 = wkp.tile([128, G, K], fp32, tag="e2")
                nc.scalar.activation(e2[:], t3[:], AF.Exp)
                nc.vector.reduce_sum(st_all[:, cols], e2[:], axis=AX.X)
                nc.vector.reduce_sum(zs_all[:, cols], zt_t[:], axis=AX.X)

            # ---- epilogue: 4 partial sums per partition ----
            o = epp.tile([128, 4], fp32, tag="o")
            lnsu = epp.tile([128, NG], fp32, tag="lnsu")
            nc.scalar.activation(lnsu[:], su_all[:], AF.Ln)
            tot = epp.tile([128, NG], fp32, tag="tot")
            nc.vector.tensor_add(tot[:], lnsu[:], mu_all[:])
            nc.vector.reduce_sum(o[:, 0:1], tot[:], axis=AX.X)
            lnsz = epp.tile([128, NG], fp32, tag="lnsz")
            nc.scalar.activation(lnsz[:], sz_all[:], AF.Ln)
            nc.vector.reduce_sum(o[:, 1:2], lnsz[:], axis=AX.X)
            lnst = epp.tile([128, NG], fp32, tag="lnst")
            nc.scalar.activation(lnst[:], st_all[:], AF.Ln)
            nc.vector.reduce_sum(o[:, 2:3], lnst[:], axis=AX.X)
            nc.vector.reduce_sum(o[:, 3:4], zs_all[:], axis=AX.X)
            nc.sync.dma_start(outp[:], o[:])

    nc.compile()
    return nc


def _prep_inputs(met_locs, mu, pi, lambda_mu, b, C, r, z):
    """Host-side packing. Returns (in_maps, host_ctx)."""
    f64 = np.float64
    mu64 = mu.astype(f64)
    r64 = r.astype(f64)
    pi64 = pi.astype(f64)

    # per-k constants
    a = -0.5 * np.exp(-r64)                       # [K]
    mu2 = (mu64 ** 2).sum(1)                      # [K]
    ck = -0.5 * D * (r64 + LOG2PI)                # [K]
    cck = a * mu2 + ck                            # [K]
    m = pi64.max()
    lnpi64 = pi64 - (m + np.log(np.exp(pi64 - m).sum()))

    # hi/lo split of the per-k constants (a_k, cck): fp16 rounding is
    # systematic across all N rows, so carry residuals on extra rows.
    rhs = np.zeros((21, K), np.float16)
    rhs[0:16, :] = (-2.0 * a[None, :] * mu64.T).astype(np.float16)
    a_hi = a.astype(np.float16)
    rhs[16, :] = a_hi
    cck_hi = cck.astype(np.float16)
    rhs[17, :] = cck_hi
    rhs[18, :] = (cck - cck_hi.astype(f64)).astype(np.float16)
    rhs[19, :] = (a - a_hi.astype(f64)).astype(np.float16)
    rhs[20, :] = a_hi                     # multiplies the x2 fp16 residual

    lnpik = np.ascontiguousarray(
        np.broadcast_to(lnpi64.astype(np.float32)[None, :], (128, K)))

    in_maps = []
    for i in range(NCORES):
        rs = slice(i * NS, (i + 1) * NS)
        xc = met_locs[rs]                          # [NS, 16] fp32
        zt3 = z[rs].astype(np.float16).reshape(NG, 128, K)

        x64 = xc.astype(f64)
        x2c = np.einsum("nd,nd->n", x64, x64)
        lpk = np.empty((21, NS), np.float16)
        lpk[0:16, :] = xc.T
        x2_hi = x2c.astype(np.float16)
        lpk[16, :] = x2_hi
        lpk[17, :] = 1.0
        lpk[18, :] = 1.0                      # carries cck_lo
        lpk[19, :] = x2_hi                    # carries a_lo
        lpk[20, :] = (x2c - x2_hi.astype(f64)).astype(np.float16)

        in_maps.append({
            "zt3": zt3,
            "lpk": lpk,
            "rhs": rhs,
            "lnpik": lnpik,
        })

    const0 = (math.lgamma(float(K)) + (K - 1) * math.log(TAU)
              + float(lnpi64.sum()))
    return in_maps, {"const0": const0, "lnpi64": lnpi64}


def _host_small_losses(met_locs, mu, pi, lambda_mu, b, C, r, lnpi64):
    """All parameter-only losses in float64, mirroring the reference."""
    f64 = np.float64
    x64 = met_locs.astype(f64)
    R = x64.max(0) - x64.min(0)
    Df = float(D)
    c = 1.25 + (D - 1) / 4.0
    g = 0.25 + (D - 1) / 4.0
    Gc = c / (50.0 * g) * math.sqrt(float((R ** 2).sum()))

    pi_loss = -((1.0 / K - 1.0) * lnpi64).sum()

    lam = lambda_mu.astype(f64)
    var_mu = (lam ** 2) * R
    mu64 = mu.astype(f64)
    b64 = b.astype(f64)
    mu_lp = (-0.5 * (((mu64 - b64) ** 2) / var_mu[None, :]).sum(1)
             - 0.5 * np.log(var_mu).sum() - 0.5 * Df * LOG2PI)
    mu_loss = -mu_lp.sum()

    lam_lp = (0.5 * math.log(0.5) - math.lgamma(0.5)
              + (0.5 - 1.0) * lam - 0.5 * np.exp(lam))
    lambda_loss = -lam_lp.sum()

    b_loss = 0.5 * (b64 ** 2).sum() + 0.5 * K * Df * LOG2PI

    r64 = r.astype(f64)
    C64 = C.astype(f64)
    r_lp = (c * np.log(C64) + (c - 1.0) * (-r64) - C64 * np.exp(-r64)
            - math.lgamma(c))
    r_loss = -r_lp.sum()

    C_lp = (g * math.log(Gc) + (g - 1.0) * (-C64) - Gc * np.exp(-C64)
            - math.lgamma(g))
    C_loss = -C_lp.sum()

    return r_loss + mu_loss + pi_loss + b_loss + lambda_loss + C_loss


def kernel(met_locs, mu, pi, lambda_mu, b, C, r, z):
    from concourse import bass_utils

    met_locs = np.asarray(met_locs, dtype=np.float32)
    mu = np.asarray(mu, dtype=np.float32)
    pi = np.asarray(pi, dtype=np.float32)
    lambda_mu = np.asarray(lambda_mu, dtype=np.float32)
    b = np.asarray(b, dtype=np.float32)
    C = np.asarray(C, dtype=np.float32)
    r = np.asarray(r, dtype=np.float32)
    z = np.asarray(z, dtype=np.float32)

    if "nc" not in _cache:
        _cache["nc"] = _build_program()
    nc = _cache["nc"]

    in_maps, ctx = _prep_inputs(met_locs, mu, pi, lambda_mu, b, C, r, z)

    trace = bool(int(os.environ.get("KERNEL_TRACE", "0")))
    res = bass_utils.run_bass_kernel_spmd(
        nc, in_maps, core_ids=list(range(NCORES)), trace=trace)
    _cache["last_results"] = res

    con_mix = 0.0
    for cm in res.results:
        o = cm["outp"].astype(np.float64)
        con_mix += (o[:, 0].sum() + 63.0 * o[:, 1].sum()
                    - 64.0 * o[:, 2].sum() - (TAU + 1.0) * o[:, 3].sum())
    con_mix += N * ctx["const0"]
    z_loss = -con_mix

    small = _host_small_losses(met_locs, mu, pi, lambda_mu, b, C, r,
                               ctx["lnpi64"])
    total = z_loss + small
    return np.asarray(total, dtype=np.float32)


# revision 6
# speedup vs baseline: 2.7108x; 1.2869x over previous
"""Trainium2 Bass kernel for nn_Clusterer loss (Concrete-mixture clustering loss).

Strategy (data-parallel over N across 8 cores, per sharding hint):
  Natural-layout design: z ships exactly once as fp16 [N, 64] (cast only, no
  host transpose); rows live on SBUF partitions, so every per-row reduction
  over K is a free-axis DVE/ACT reduction:
    v = z + logN:   PE matmul of a 21-row x-feature pack (x^T, x2 hi/lo, ones)
                    against per-k constants gives the logN part in PSUM; one
                    scalar_tensor_tensor adds z.
    row stats:      max_k v, sum e^{v-max}, sum e^z, sum pi_k e^{-tau z},
                    sum z -- all AX.X reduces into [128, NG] stat tiles.
  Per-core output = 4 partial sums [128, 4]; host combines in float64:
    con+mix = const0 + (M + ln su) + 63*ln sz - 64*ln st - 1.1*sum z.
  Tiny K/D-sized losses (pi/mu/lambda/b/r/C) computed on host in float64.
"""

import math
import os

import ml_dtypes
import numpy as np

_FP8 = ml_dtypes.float8_e4m3

N, D, K = 262144, 16, 64
NCORES = 8
NS = N // NCORES          # rows per core = 32768
NG = NS // 128            # 128-row tiles per core = 256
G = 16                    # tiles per chunk
NCH = NG // G             # chunks = 16
FD = G * K                # free dim per chunk = 1024
TAU = 0.1
LOG2PI = math.log(2.0 * math.pi)

_cache = {}


def _build_program():
    import concourse.bacc as bacc
    import concourse.mybir as mybir
    import concourse.tile as tile

    fp16 = mybir.dt.float16
    fp32 = mybir.dt.float32
    fp8 = mybir.dt.float8e4
    AF = mybir.ActivationFunctionType
    ALU = mybir.AluOpType
    AX = mybir.AxisListType

    nc = bacc.Bacc("TRN2", target_bir_lowering=False, debug=False,
                   num_devices=NCORES)

    zt3 = nc.dram_tensor("zt3", [NG, 128, K], fp8, kind="ExternalInput").ap()
    lpk = nc.dram_tensor("lpk", [21, NS], fp16, kind="ExternalInput").ap()
    rhs = nc.dram_tensor("rhs", [21, K], fp16, kind="ExternalInput").ap()
    lnpik = nc.dram_tensor("lnpik", [128, K], fp32, kind="ExternalInput").ap()
    outp = nc.dram_tensor("outp", [128, 4], fp32, kind="ExternalOutput").ap()

    with tile.TileContext(nc) as tc:
        with (
            tc.tile_pool(name="const", bufs=1) as constp,
            tc.tile_pool(name="stats", bufs=1) as statp,
            tc.tile_pool(name="lp", bufs=3) as lpp,
            tc.tile_pool(name="zt", bufs=3) as ztp,
            tc.tile_pool(name="wk", bufs=2) as wkp,
            tc.tile_pool(name="ep", bufs=1) as epp,
            tc.tile_pool(name="ps", bufs=2, space="PSUM") as psp,
        ):
            rhs_t = constp.tile([21, K], fp16, tag="rhs")
            nc.sync.dma_start(rhs_t[:], rhs[:])
            lnpik_t = constp.tile([128, K], fp32, tag="lnpik")
            nc.sync.dma_start(lnpik_t[:], lnpik[:])
            # replicate lnpi along the chunk axis once: [128, G, K]
            lnpirep = constp.tile([128, G, K], fp32, tag="lnpirep")
            for g in range(G):
                nc.scalar.activation(lnpirep[:, g, :], lnpik_t[:], AF.Copy)

            mu_all = statp.tile([128, NG], fp32, tag="mu_all")
            su_all = statp.tile([128, NG], fp32, tag="su_all")
            sz_all = statp.tile([128, NG], fp32, tag="sz_all")
            st_all = statp.tile([128, NG], fp32, tag="st_all")
            zs_all = statp.tile([128, NG], fp32, tag="zs_all")

            for c in range(NCH):
                cols = slice(c * G, (c + 1) * G)
                lp_t = lpp.tile([21, G * 128], fp16, tag="lp")
                nc.sync.dma_start(
                    lp_t[:], lpk[:, c * G * 128:(c + 1) * G * 128])
                zt_t = ztp.tile([128, G, K], fp8, tag="zt")
                nc.sync.dma_start(
                    zt_t[:],
                    zt3[c * G:(c + 1) * G].rearrange("g p k -> p g k"))
                z32 = wkp.tile([128, G, K], fp32, tag="z32")
                nc.scalar.activation(z32[:], zt_t[:], AF.Copy)

                ps = psp.tile([128, FD], fp32, tag="ps")
                for g in range(G):
                    nc.tensor.matmul(
                        ps[:, g * K:(g + 1) * K],
                        lhsT=lp_t[:, g * 128:(g + 1) * 128],
                        rhs=rhs_t[:],
                        start=True, stop=True,
                    )
                ps3 = ps[:].rearrange("p (g k) -> p g k", k=K)

                # v = z + logN
                v = wkp.tile([128, G, K], fp32, tag="v")
                nc.vector.scalar_tensor_tensor(
                    v[:], in0=z32[:], scalar=1.0, in1=ps3,
                    op0=ALU.mult, op1=ALU.add)
                mu_sl = mu_all[:, cols]
                nc.vector.reduce_max(mu_sl, v[:], axis=AX.X)
                vc = wkp.tile([128, G, K], fp32, tag="vc")
                nc.vector.scalar_tensor_tensor(
                    vc[:], in0=v[:], scalar=1.0,
                    in1=mu_sl.broadcast_to([128, G, K]),
                    op0=ALU.mult, op1=ALU.subtract)
                e = wkp.tile([128, G, K], fp32, tag="e")
                nc.scalar.activation(e[:], vc[:], AF.Exp)
                nc.vector.reduce_sum(su_all[:, cols], e[:], axis=AX.X)

                # con-side sums from the same natural-layout z tile
                e1 = wkp.tile([128, G, K], fp32, tag="e1")
                nc.scalar.activation(e1[:], z32[:], AF.Exp)
                nc.vector.reduce_sum(sz_all[:, cols], e1[:], axis=AX.X)
                t3 = wkp.tile([128, G, K], fp32, tag="t3")
                nc.vector.scalar_tensor_tensor(
                    t3[:], in0=z32[:], scalar=-TAU, in1=lnpirep[:],
                    op0=ALU.mult, op1=ALU.add)
                e2 = wkp.tile([128, G, K], fp32, tag="e2")
                nc.scalar.activation(e2[:], t3[:], AF.Exp)
                nc.vector.reduce_sum(st_all[:, cols], e2[:], axis=AX.X)
                nc.vector.reduce_sum(zs_all[:, cols], z32[:], axis=AX.X)

            # ---- epilogue: 4 partial sums per partition ----
            o = epp.tile([128, 4], fp32, tag="o")
            lnsu = epp.tile([128, NG], fp32, tag="lnsu")
            nc.scalar.activation(lnsu[:], su_all[:], AF.Ln)
            tot = epp.tile([128, NG], fp32, tag="tot")
            nc.vector.tensor_add(tot[:], lnsu[:], mu_all[:])
            nc.vector.reduce_sum(o[:, 0:1], tot[:], axis=AX.X)
            lnsz = epp.tile([128, NG], fp32, tag="lnsz")
            nc.scalar.activation(lnsz[:], sz_all[:], AF.Ln)
            nc.vector.reduce_sum(o[:, 1:2], lnsz[:], axis=AX.X)
            lnst = epp.tile([128, NG], fp32, tag="lnst")
            nc.scalar.activation(lnst[:], st_all[:], AF.Ln)
            nc.vector.reduce_sum(o[:, 2:3], lnst[:], axis=AX.X)
            nc.vector.reduce_sum(o[:, 3:4], zs_all[:], axis=AX.X)
            nc.sync.dma_start(outp[:], o[:])

    nc.compile()
    return nc


def _prep_inputs(met_locs, mu, pi, lambda_mu, b, C, r, z):
    """Host-side packing. Returns (in_maps, host_ctx)."""
    f64 = np.float64
    mu64 = mu.astype(f64)
    r64 = r.astype(f64)
    pi64 = pi.astype(f64)

    # per-k constants
    a = -0.5 * np.exp(-r64)                       # [K]
    mu2 = (mu64 ** 2).sum(1)                      # [K]
    ck = -0.5 * D * (r64 + LOG2PI)                # [K]
    cck = a * mu2 + ck                            # [K]
    m = pi64.max()
    lnpi64 = pi64 - (m + np.log(np.exp(pi64 - m).sum()))

    # hi/lo split of the per-k constants (a_k, cck): fp16 rounding is
    # systematic across all N rows, so carry residuals on extra rows.
    rhs = np.zeros((21, K), np.float16)
    rhs[0:16, :] = (-2.0 * a[None, :] * mu64.T).astype(np.float16)
    a_hi = a.astype(np.float16)
    rhs[16, :] = a_hi
    cck_hi = cck.astype(np.float16)
    rhs[17, :] = cck_hi
    rhs[18, :] = (cck - cck_hi.astype(f64)).astype(np.float16)
    rhs[19, :] = (a - a_hi.astype(f64)).astype(np.float16)
    rhs[20, :] = a_hi                     # multiplies the x2 fp16 residual

    lnpik = np.ascontiguousarray(
        np.broadcast_to(lnpi64.astype(np.float32)[None, :], (128, K)))

    in_maps = []
    for i in range(NCORES):
        rs = slice(i * NS, (i + 1) * NS)
        xc = met_locs[rs]                          # [NS, 16] fp32
        zt3 = z[rs].astype(_FP8).reshape(NG, 128, K)

        x64 = xc.astype(f64)
        x2c = np.einsum("nd,nd->n", x64, x64)
        lpk = np.empty((21, NS), np.float16)
        lpk[0:16, :] = xc.T
        x2_hi = x2c.astype(np.float16)
        lpk[16, :] = x2_hi
        lpk[17, :] = 1.0
        lpk[18, :] = 1.0                      # carries cck_lo
        lpk[19, :] = x2_hi                    # carries a_lo
        lpk[20, :] = (x2c - x2_hi.astype(f64)).astype(np.float16)

        in_maps.append({
            "zt3": zt3,
            "lpk": lpk,
            "rhs": rhs,
            "lnpik": lnpik,
        })

    const0 = (math.lgamma(float(K)) + (K - 1) * math.log(TAU)
              + float(lnpi64.sum()))
    return in_maps, {"const0": const0, "lnpi64": lnpi64}


def _host_small_losses(met_locs, mu, pi, lambda_mu, b, C, r, lnpi64):
    """All parameter-only losses in float64, mirroring the reference."""
    f64 = np.float64
    x64 = met_locs.astype(f64)
    R = x64.max(0) - x64.min(0)
    Df = float(D)
    c = 1.25 + (D - 1) / 4.0
    g = 0.25 + (D - 1) / 4.0
    Gc = c / (50.0 * g) * math.sqrt(float((R ** 2).sum()))

    pi_loss = -((1.0 / K - 1.0) * lnpi64).sum()

    lam = lambda_mu.astype(f64)
    var_mu = (lam ** 2) * R
    mu64 = mu.astype(f64)
    b64 = b.astype(f64)
    mu_lp = (-0.5 * (((mu64 - b64) ** 2) / var_mu[None, :]).sum(1)
             - 0.5 * np.log(var_mu).sum() - 0.5 * Df * LOG2PI)
    mu_loss = -mu_lp.sum()

    lam_lp = (0.5 * math.log(0.5) - math.lgamma(0.5)
              + (0.5 - 1.0) * lam - 0.5 * np.exp(lam))
    lambda_loss = -lam_lp.sum()

    b_loss = 0.5 * (b64 ** 2).sum() + 0.5 * K * Df * LOG2PI

    r64 = r.astype(f64)
    C64 = C.astype(f64)
    r_lp = (c * np.log(C64) + (c - 1.0) * (-r64) - C64 * np.exp(-r64)
            - math.lgamma(c))
    r_loss = -r_lp.sum()

    C_lp = (g * math.log(Gc) + (g - 1.0) * (-C64) - Gc * np.exp(-C64)
            - math.lgamma(g))
    C_loss = -C_lp.sum()

    return r_loss + mu_loss + pi_loss + b_loss + lambda_loss + C_loss


def kernel(met_locs, mu, pi, lambda_mu, b, C, r, z):
    from concourse import bass_utils

    met_locs = np.asarray(met_locs, dtype=np.float32)
    mu = np.asarray(mu, dtype=np.float32)
    pi = np.asarray(pi, dtype=np.float32)
    lambda_mu = np.asarray(lambda_mu, dtype=np.float32)
    b = np.asarray(b, dtype=np.float32)
    C = np.asarray(C, dtype=np.float32)
    r = np.asarray(r, dtype=np.float32)
    z = np.asarray(z, dtype=np.float32)

    if "nc" not in _cache:
        _cache["nc"] = _build_program()
    nc = _cache["nc"]

    in_maps, ctx = _prep_inputs(met_locs, mu, pi, lambda_mu, b, C, r, z)

    trace = bool(int(os.environ.get("KERNEL_TRACE", "0")))
    res = bass_utils.run_bass_kernel_spmd(
        nc, in_maps, core_ids=list(range(NCORES)), trace=trace)
    _cache["last_results"] = res

    con_mix = 0.0
    for cm in res.results:
        o = cm["outp"].astype(np.float64)
        con_mix += (o[:, 0].sum() + 63.0 * o[:, 1].sum()
                    - 64.0 * o[:, 2].sum() - (TAU + 1.0) * o[:, 3].sum())
    con_mix += N * ctx["const0"]
    z_loss = -con_mix

    small = _host_small_losses(met_locs, mu, pi, lambda_mu, b, C, r,
                               ctx["lnpi64"])
    total = z_loss + small
    return np.asarray(total, dtype=np.float32)


# revision 8
# speedup vs baseline: 3.0705x; 1.1327x over previous
"""Trainium2 Bass kernel for nn_Clusterer loss (Concrete-mixture clustering loss).

Strategy (data-parallel over N across 8 cores, per sharding hint):
  Natural-layout design: z ships exactly once as fp8-e4m3 [N, 64] (cast only,
  no host transpose); rows live on SBUF partitions, so every per-row reduction
  over K is a free-axis DVE/ACT reduction:
    v = z + logN:   PE fp8 matmul x^T @ w gives the cross term in PSUM;
                    a_k*x2 + cck_k added on DVE in f32 (x2 shipped exact f32,
                    per-k consts replicated across partitions on device).
    row stats:      max_k v, sum e^{v-max}, sum e^z, sum pi_k e^{-tau z},
                    sum z -- all AX.X reduces into [128, NG] stat tiles.
  Shipped per core: z fp8 [NG,128,64], x^T fp8 [16, NS]+w cols, aux f32
  (x2 tile-major + lnpi/cck/a rows). Per-core output = 4 partial sums
  [128, 4]; host combines in float64:
    con+mix = const0 + (M + ln su) + 63*ln sz - 64*ln st - 1.1*sum z.
  Tiny K/D-sized losses (pi/mu/lambda/b/r/C) computed on host in float64.
"""

import math
import os

import ml_dtypes
import numpy as np

_FP8 = ml_dtypes.float8_e4m3

N, D, K = 262144, 16, 64
NCORES = 8
NS = N // NCORES          # rows per core = 32768
NG = NS // 128            # 128-row tiles per core = 256
G = 16                    # tiles per chunk
NCH = NG // G             # chunks = 16
FD = G * K                # free dim per chunk = 1024
AUXC = NG + 3 * K         # aux cols: x2 tile-major ++ lnpi ++ cck ++ a
TAU = 0.1
LOG2PI = math.log(2.0 * math.pi)

_cache = {}

# fp16 -> fp8 cast LUT (double rounding only moves exact ties; harmless here)
with np.errstate(invalid="ignore", over="ignore"):
    _LUT8 = (np.arange(65536, dtype=np.uint16).view(np.float16)
             .astype(_FP8).view(np.uint8))


def _to_fp8(a16):
    return _LUT8[a16.view(np.uint16)].view(_FP8)


def _build_program():
    import concourse.bacc as bacc
    import concourse.mybir as mybir
    import concourse.tile as tile

    fp32 = mybir.dt.float32
    fp8 = mybir.dt.float8e4
    AF = mybir.ActivationFunctionType
    ALU = mybir.AluOpType
    AX = mybir.AxisListType

    nc = bacc.Bacc("TRN2", target_bir_lowering=False, debug=False,
                   num_devices=NCORES)

    zt3 = nc.dram_tensor("zt3", [NG, 128, K], fp8, kind="ExternalInput").ap()
    lpk8 = nc.dram_tensor("lpk8", [16, NS + K], fp8,
                          kind="ExternalInput").ap()
    aux = nc.dram_tensor("aux", [128, AUXC], fp32, kind="ExternalInput").ap()
    outp = nc.dram_tensor("outp", [128, 4], fp32, kind="ExternalOutput").ap()

    with tile.TileContext(nc) as tc:
        with (
            tc.tile_pool(name="const", bufs=1) as constp,
            tc.tile_pool(name="stats", bufs=1) as statp,
            tc.tile_pool(name="lp", bufs=3) as lpp,
            tc.tile_pool(name="zt", bufs=3) as ztp,
            tc.tile_pool(name="wk", bufs=2) as wkp,
            tc.tile_pool(name="ep", bufs=1) as epp,
            tc.tile_pool(name="ps", bufs=2, space="PSUM") as psp,
        ):
            rhs_t = constp.tile([16, K], fp8, tag="rhs")
            nc.sync.dma_start(rhs_t[:], lpk8[:, NS:NS + K])
            aux_t = constp.tile([128, AUXC], fp32, tag="aux")
            nc.sync.dma_start(aux_t[:], aux[:])
            # replicate per-k const rows along the chunk axis: [128, G, K]
            lnpirep = constp.tile([128, G, K], fp32, tag="lnpirep")
            cckrep = constp.tile([128, G, K], fp32, tag="cckrep")
            areprep = constp.tile([128, G, K], fp32, tag="areprep")
            for g in range(G):
                nc.scalar.activation(lnpirep[:, g, :],
                                     aux_t[:, NG:NG + K], AF.Copy)
                nc.scalar.activation(cckrep[:, g, :],
                                     aux_t[:, NG + K:NG + 2 * K], AF.Copy)
                nc.scalar.activation(areprep[:, g, :],
                                     aux_t[:, NG + 2 * K:NG + 3 * K], AF.Copy)

            mu_all = statp.tile([128, NG], fp32, tag="mu_all")
            su_all = statp.tile([128, NG], fp32, tag="su_all")
            sz_all = statp.tile([128, NG], fp32, tag="sz_all")
            st_all = statp.tile([128, NG], fp32, tag="st_all")
            zs_all = statp.tile([128, NG], fp32, tag="zs_all")

            for c in range(NCH):
                cols = slice(c * G, (c + 1) * G)
                lp_t = lpp.tile([16, G * 128], fp8, tag="lp")
                nc.sync.dma_start(
                    lp_t[:], lpk8[:, c * G * 128:(c + 1) * G * 128])
                zt_t = ztp.tile([128, G, K], fp8, tag="zt")
                nc.sync.dma_start(
                    zt_t[:],
                    zt3[c * G:(c + 1) * G].rearrange("g p k -> p g k"))
                z32 = wkp.tile([128, G, K], fp32, tag="z32")
                nc.scalar.activation(z32[:], zt_t[:], AF.Copy)

                ps = psp.tile([128, FD], fp32, tag="ps")
                for g in range(G):
                    nc.tensor.matmul(
                        ps[:, g * K:(g + 1) * K],
                        lhsT=lp_t[:, g * 128:(g + 1) * 128],
                        rhs=rhs_t[:],
                        start=True, stop=True,
                    )
                ps3 = ps[:].rearrange("p (g k) -> p g k", k=K)

                # logN constant part: t4 = a_k * x2 + cck_k
                x2b = aux_t[:, cols].broadcast_to([128, G, K])
                t4 = wkp.tile([128, G, K], fp32, tag="t4")
                nc.vector.tensor_tensor(t4[:], x2b, areprep[:],
                                        op=ALU.mult)
                nc.vector.tensor_add(t4[:], t4[:], cckrep[:])
                # v = z + w.x + t4
                v = wkp.tile([128, G, K], fp32, tag="v")
                nc.vector.scalar_tensor_tensor(
                    v[:], in0=z32[:], scalar=1.0, in1=ps3,
                    op0=ALU.mult, op1=ALU.add)
                nc.vector.tensor_add(v[:], v[:], t4[:])
                mu_sl = mu_all[:, cols]
                nc.vector.reduce_max(mu_sl, v[:], axis=AX.X)
                vc = wkp.tile([128, G, K], fp32, tag="vc")
                nc.vector.scalar_tensor_tensor(
                    vc[:], in0=v[:], scalar=1.0,
                    in1=mu_sl.broadcast_to([128, G, K]),
                    op0=ALU.mult, op1=ALU.subtract)
                e = wkp.tile([128, G, K], fp32, tag="e")
                nc.scalar.activation(e[:], vc[:], AF.Exp)
                nc.vector.reduce_sum(su_all[:, cols], e[:], axis=AX.X)

                # con-side sums from the same natural-layout z tile
                e1 = wkp.tile([128, G, K], fp32, tag="e1")
                nc.scalar.activation(e1[:], z32[:], AF.Exp)
                nc.vector.reduce_sum(sz_all[:, cols], e1[:], axis=AX.X)
                t3 = wkp.tile([128, G, K], fp32, tag="t3")
                nc.vector.scalar_tensor_tensor(
                    t3[:], in0=z32[:], scalar=-TAU, in1=lnpirep[:],
                    op0=ALU.mult, op1=ALU.add)
                e2 = wkp.tile([128, G, K], fp32, tag="e2")
                nc.scalar.activation(e2[:], t3[:], AF.Exp)
                nc.vector.reduce_sum(st_all[:, cols], e2[:], axis=AX.X)
                nc.vector.reduce_sum(zs_all[:, cols], z32[:], axis=AX.X)

            # ---- epilogue: 4 partial sums per partition ----
            o = epp.tile([128, 4], fp32, tag="o")
            lnsu = epp.tile([128, NG], fp32, tag="lnsu")
            nc.scalar.activation(lnsu[:], su_all[:], AF.Ln)
            tot = epp.tile([128, NG], fp32, tag="tot")
            nc.vector.tensor_add(tot[:], lnsu[:], mu_all[:])
            nc.vector.reduce_sum(o[:, 0:1], tot[:], axis=AX.X)
            lnsz = epp.tile([128, NG], fp32, tag="lnsz")
            nc.scalar.activation(lnsz[:], sz_all[:], AF.Ln)
            nc.vector.reduce_sum(o[:, 1:2], lnsz[:], axis=AX.X)
            lnst = epp.tile([128, NG], fp32, tag="lnst")
            nc.scalar.activation(lnst[:], st_all[:], AF.Ln)
            nc.vector.reduce_sum(o[:, 2:3], lnst[:], axis=AX.X)
            nc.vector.reduce_sum(o[:, 3:4], zs_all[:], axis=AX.X)
            nc.sync.dma_start(outp[:], o[:])

    nc.compile()
    return nc


def _prep_inputs(met_locs, mu, pi, lambda_mu, b, C, r, z):
    """Host-side packing. Returns (in_maps, host_ctx)."""
    f64 = np.float64
    mu64 = mu.astype(f64)
    r64 = r.astype(f64)
    pi64 = pi.astype(f64)

    # per-k constants
    a = -0.5 * np.exp(-r64)                       # [K]
    mu2 = (mu64 ** 2).sum(1)                      # [K]
    ck = -0.5 * D * (r64 + LOG2PI)                # [K]
    cck = a * mu2 + ck                            # [K]
    m = pi64.max()
    lnpi64 = pi64 - (m + np.log(np.exp(pi64 - m).sum()))

    with np.errstate(invalid="ignore", over="ignore"):
        w8 = np.ascontiguousarray(
            (-2.0 * a[None, :] * mu64.T)).astype(_FP8)   # [16, K]
        xT8 = _to_fp8(met_locs.T.astype(np.float16))     # [16, N]

    consts = np.empty((3 * K,), np.float32)
    consts[0:K] = lnpi64
    consts[K:2 * K] = cck
    consts[2 * K:3 * K] = a
    const_rows = np.broadcast_to(consts[None, :], (128, 3 * K))

    x2_all = np.einsum("nd,nd->n", met_locs, met_locs,
                       dtype=f64)                        # [N] exact-ish

    in_maps = []
    for i in range(NCORES):
        rs = slice(i * NS, (i + 1) * NS)
        zt3 = _to_fp8(z[rs].astype(np.float16)).reshape(NG, 128, K)

        lpk8 = np.empty((16, NS + K), _FP8)
        lpk8[:, 0:NS] = xT8[:, rs]
        lpk8[:, NS:] = w8

        aux = np.empty((128, AUXC), np.float32)
        aux[:, 0:NG] = x2_all[rs].reshape(NG, 128).T
        aux[:, NG:] = const_rows

        in_maps.append({"zt3": zt3, "lpk8": lpk8, "aux": aux})

    const0 = (math.lgamma(float(K)) + (K - 1) * math.log(TAU)
              + float(lnpi64.sum()))
    return in_maps, {"const0": const0, "lnpi64": lnpi64}


def _host_small_losses(met_locs, mu, pi, lambda_mu, b, C, r, lnpi64):
    """All parameter-only losses in float64, mirroring the reference."""
    f64 = np.float64
    x64 = met_locs.astype(f64)
    R = x64.max(0) - x64.min(0)
    Df = float(D)
    c = 1.25 + (D - 1) / 4.0
    g = 0.25 + (D - 1) / 4.0
    Gc = c / (50.0 * g) * math.sqrt(float((R ** 2).sum()))

    pi_loss = -((1.0 / K - 1.0) * lnpi64).sum()

    lam = lambda_mu.astype(f64)
    var_mu = (lam ** 2) * R
    mu64 = mu.astype(f64)
    b64 = b.astype(f64)
    mu_lp = (-0.5 * (((mu64 - b64) ** 2) / var_mu[None, :]).sum(1)
             - 0.5 * np.log(var_mu).sum() - 0.5 * Df * LOG2PI)
    mu_loss = -mu_lp.sum()

    lam_lp = (0.5 * math.log(0.5) - math.lgamma(0.5)
              + (0.5 - 1.0) * lam - 0.5 * np.exp(lam))
    lambda_loss = -lam_lp.sum()

    b_loss = 0.5 * (b64 ** 2).sum() + 0.5 * K * Df * LOG2PI

    r64 = r.astype(f64)
    C64 = C.astype(f64)
    r_lp = (c * np.log(C64) + (c - 1.0) * (-r64) - C64 * np.exp(-r64)
            - math.lgamma(c))
    r_loss = -r_lp.sum()

    C_lp = (g * math.log(Gc) + (g - 1.0) * (-C64) - Gc * np.exp(-C64)
            - math.lgamma(g))
    C_loss = -C_lp.sum()

    return r_loss + mu_loss + pi_loss + b_loss + lambda_loss + C_loss


def kernel(met_locs, mu, pi, lambda_mu, b, C, r, z):
    from concourse import bass_utils

    met_locs = np.asarray(met_locs, dtype=np.float32)
    mu = np.asarray(mu, dtype=np.float32)
    pi = np.asarray(pi, dtype=np.float32)
    lambda_mu = np.asarray(lambda_mu, dtype=np.float32)
    b = np.asarray(b, dtype=np.float32)
    C = np.asarray(C, dtype=np.float32)
    r = np.asarray(r, dtype=np.float32)
    z = np.asarray(z, dtype=np.float32)

    if "nc" not in _cache:
        _cache["nc"] = _build_program()
    nc = _cache["nc"]

    in_maps, ctx = _prep_inputs(met_locs, mu, pi, lambda_mu, b, C, r, z)

    trace = bool(int(os.environ.get("KERNEL_TRACE", "0")))
    res = bass_utils.run_bass_kernel_spmd(
        nc, in_maps, core_ids=list(range(NCORES)), trace=trace)
    _cache["last_results"] = res

    con_mix = 0.0
    for cm in res.results:
        o = cm["outp"].astype(np.float64)
        con_mix += (o[:, 0].sum() + 63.0 * o[:, 1].sum()
                    - 64.0 * o[:, 2].sum() - (TAU + 1.0) * o[:, 3].sum())
    con_mix += N * ctx["const0"]
    z_loss = -con_mix

    small = _host_small_losses(met_locs, mu, pi, lambda_mu, b, C, r,
                               ctx["lnpi64"])
    total = z_loss + small
    return np.asarray(total, dtype=np.float32)
